# revision 4
# baseline (speedup 1.0000x reference)
"""Trainium2 Bass kernel for nn_DisentangledHead (disentangled attention head).

Reference computation (per batch element b):
    q_c = content[b] @ w_qc ; k_c = content[b] @ w_kc ; v = content[b] @ w_v
    q_p = position @ w_qp   ; k_p = position @ w_kp
    S   = (q_c k_c^T + q_p k_p^T) * scale          [T, T]
    attn = softmax(S, -1)                           [T, T]
    out  = attn @ v                                 [T, H]
Returns (out [B,T,H], attn [B,T,T]).

Sharding: data-parallel over B across the 8 NeuronCores (1 batch element
per core; position + weights replicated).

Kernel strategy per core:
  - Build content^T / position^T via PE transposes (2 row-tiles per 128x128
    transpose).
  - Project to q_fullT / k_fullT [128, T]: partitions 0:64 hold the content
    projection transposed, 64:128 the position projection transposed, so a
    single K=128 matmul computes q_c k_c^T + q_p k_p^T.
  - Pass A (q-major): S tile [128q, T] on PE -> ACT exp(scale*S) with
    accum_out row-sums -> DVE reciprocal + tensor_scalar normalize ->
    DMA attn rows out.
  - Pass B (k-major): S^T tiles -> ACT exp -> PE accumulates
    out^T = v^T @ exp(S^T) with v stationary -> PE re-transpose ->
    DVE scale by 1/rowsum -> DMA out.
"""

import numpy as np
from contextlib import ExitStack

import concourse.bass as bass
import concourse.tile as tile
from concourse import bacc, mybir
from concourse.bass_utils import run_bass_kernel_spmd
from concourse.masks import make_identity

F32 = mybir.dt.float32
AF = mybir.ActivationFunctionType

B = 8
T = 2048
C = 64
H = 64
P = 128
NT = T // P  # 16
NCORES = 8
SCALE = 1.0 / 8.0  # H ** -0.5

_INPUT_SPECS = [
    ("content", [T, C]),
    ("position", [T, C]),
    ("w_qc", [C, H]),
    ("w_kc", [C, H]),
    ("w_v", [C, H]),
    ("w_qp", [C, H]),
    ("w_kp", [C, H]),
]


def _emit(tc, ins, out_d, attn_d):
    nc = tc.nc
    with ExitStack() as ctx:
        consts = ctx.enter_context(tc.tile_pool(name="consts", bufs=1))
        persist = ctx.enter_context(tc.tile_pool(name="persist", bufs=1))

        ident = consts.tile([P, P], F32)
        make_identity(nc, ident)

        w_sb = {}
        for wname in ("w_qc", "w_kc", "w_v", "w_qp", "w_kp"):
            w_sb[wname] = consts.tile([C, H], F32, name=f"w_{wname}")
            nc.sync.dma_start(out=w_sb[wname], in_=ins[wname])

        # ---- content^T / position^T ------------------------------------
        xT = persist.tile([C, T], F32)  # content^T
        pT = persist.tile([C, T], F32)  # position^T

        with tc.tile_pool(name="tr_in", bufs=4) as tr_in, \
             tc.tile_pool(name="tr_ps", bufs=4, space="PSUM") as tr_ps:
            for src, dstT in ((ins["content"], xT), (ins["position"], pT)):
                for g in range(NT // 2):
                    nat = tr_in.tile([P, 2, C], F32)
                    # rows [256g, 256g+256) as [p, j, c] with t = j*128 + p
                    nc.sync.dma_start(
                        out=nat,
                        in_=src[2 * g * P:(2 * g + 2) * P, :].rearrange(
                            "(j p) c -> p j c", j=2))
                    pst = tr_ps.tile([P, P], F32)
                    nc.tensor.transpose(pst, nat, ident)
                    nc.vector.tensor_copy(
                        dstT[:, (2 * g) * P:(2 * g + 1) * P], pst[0:C, :])
                    nc.vector.tensor_copy(
                        dstT[:, (2 * g + 1) * P:(2 * g + 2) * P], pst[C:2 * C, :])

        # ---- projections ------------------------------------------------
        # qT rows 0:64 = (content @ w_qc)^T, rows 64:128 = (position @ w_qp)^T
        qT = persist.tile([P, T], F32)
        kT = persist.tile([P, T], F32)
        v_sb = persist.tile([P, NT, H], F32)  # v rows t=128*i+p at [p, i, :]

        with tc.tile_pool(name="pj_ps", bufs=4, space="PSUM") as pj_ps:
            for n in range(4):
                sl = slice(512 * n, 512 * (n + 1))
                for wname, srcT, dst in (
                    ("w_qc", xT, qT[0:C, sl]),
                    ("w_qp", pT, qT[C:P, sl]),
                    ("w_kc", xT, kT[0:C, sl]),
                    ("w_kp", pT, kT[C:P, sl]),
                ):
                    psp = pj_ps.tile([C, 512], F32, tag="psp")
                    nc.tensor.matmul(psp, lhsT=w_sb[wname], rhs=srcT[:, sl],
                                     start=True, stop=True)
                    nc.vector.tensor_copy(dst, psp)
            for i in range(NT):
                psv = pj_ps.tile([P, H], F32, tag="psv")
                nc.tensor.matmul(psv, lhsT=xT[:, P * i:P * (i + 1)],
                                 rhs=w_sb["w_v"], start=True, stop=True)
                nc.vector.tensor_copy(v_sb[:, i, :], psv)

        # ---- pass A: q-major scores -> attn ----------------------------
        sums = persist.tile([P, NT], F32)
        rsum = persist.tile([P, NT], F32)

        with tc.tile_pool(name="ps_s", bufs=2, space="PSUM") as ps_s, \
             tc.tile_pool(name="attn_sb", bufs=3) as attn_pool:
            for i in range(NT):
                ps = ps_s.tile([P, T], F32)
                for n in range(4):
                    nc.tensor.matmul(
                        ps[:, 512 * n:512 * (n + 1)],
                        lhsT=qT[:, P * i:P * (i + 1)],
                        rhs=kT[:, 512 * n:512 * (n + 1)],
                        start=True, stop=True)
                et = attn_pool.tile([P, T], F32)
                nc.scalar.activation(et, ps, AF.Exp, scale=SCALE,
                                     accum_out=sums[:, i:i + 1])
                nc.vector.reciprocal(rsum[:, i:i + 1], sums[:, i:i + 1])
                nc.vector.tensor_scalar_mul(et, et, rsum[:, i:i + 1])
                nc.sync.dma_start(out=attn_d[P * i:P * (i + 1), :], in_=et)

        # ---- pass B: k-major scores -> out -----------------------------
        oT_sb = persist.tile([H, T], F32)
        with tc.tile_pool(name="ps_st", bufs=2, space="PSUM") as ps_st, \
             tc.tile_pool(name="ps_ot", bufs=1, space="PSUM") as ps_ot_pool, \
             tc.tile_pool(name="est_sb", bufs=2) as est_pool:
            ps_ot = ps_ot_pool.tile([H, T], F32)
            for j in range(NT):
                for hh in range(2):
                    ps2 = ps_st.tile([P, T // 2], F32)
                    for n in range(2):
                        qs = slice(1024 * hh + 512 * n, 1024 * hh + 512 * (n + 1))
                        nc.tensor.matmul(
                            ps2[:, 512 * n:512 * (n + 1)],
                            lhsT=kT[:, P * j:P * (j + 1)],
                            rhs=qT[:, qs], start=True, stop=True)
                    est = est_pool.tile([P, T // 2], F32)
                    nc.scalar.activation(est, ps2, AF.Exp, scale=SCALE)
                    for n in range(2):
                        qs = slice(1024 * hh + 512 * n, 1024 * hh + 512 * (n + 1))
                        nc.tensor.matmul(
                            ps_ot[:, qs], lhsT=v_sb[:, j, :],
                            rhs=est[:, 512 * n:512 * (n + 1)],
                            start=(j == 0), stop=(j == NT - 1))
            nc.vector.tensor_copy(oT_sb, ps_ot)

        # out^T [H, T] -> out [T, H], scaled by 1/rowsum
        with tc.tile_pool(name="tr2_ps", bufs=4, space="PSUM") as tr2_ps, \
             tc.tile_pool(name="out_pool", bufs=1) as out_pool:
            out_sb = out_pool.tile([P, NT, H], F32)
            for i in range(NT):
                pst2 = tr2_ps.tile([P, H], F32)
                nc.tensor.transpose(pst2, oT_sb[:, P * i:P * (i + 1)],
                                    ident[0:C, 0:C])
                nc.vector.tensor_scalar_mul(out_sb[:, i, :], pst2,
                                            rsum[:, i:i + 1])
            out_view = out_d.rearrange("(i p) h -> p i h", p=P)
            nc.sync.dma_start(out=out_view, in_=out_sb)


def build_program():
    nc = bacc.Bacc("TRN2", target_bir_lowering=False, debug=False,
                   num_devices=NCORES)
    ins = {}
    for name, shape in _INPUT_SPECS:
        ins[name] = nc.dram_tensor(name, shape, F32, kind="ExternalInput").ap()
    attn_d = nc.dram_tensor("attn", [T, T], F32, kind="ExternalOutput").ap()
    out_d = nc.dram_tensor("out", [T, H], F32, kind="ExternalOutput").ap()
    with tile.TileContext(nc) as tc:
        _emit(tc, ins, out_d, attn_d)
    nc.compile()
    return nc


_PROGRAM = None


def _get_program():
    global _PROGRAM
    if _PROGRAM is None:
        _PROGRAM = build_program()
    return _PROGRAM


def make_in_maps(content, position, w_qc, w_kc, w_v, w_qp, w_kp):
    common = {
        "position": np.ascontiguousarray(position, dtype=np.float32),
        "w_qc": np.ascontiguousarray(w_qc, dtype=np.float32),
        "w_kc": np.ascontiguousarray(w_kc, dtype=np.float32),
        "w_v": np.ascontiguousarray(w_v, dtype=np.float32),
        "w_qp": np.ascontiguousarray(w_qp, dtype=np.float32),
        "w_kp": np.ascontiguousarray(w_kp, dtype=np.float32),
    }
    return [
        {"content": np.ascontiguousarray(content[b], dtype=np.float32), **common}
        for b in range(B)
    ]


def run(inputs, trace=False):
    nc = _get_program()
    in_maps = make_in_maps(**{k: np.asarray(v) for k, v in inputs.items()})
    res = run_bass_kernel_spmd(nc, in_maps, list(range(NCORES)), trace=trace)
    out = np.stack([np.asarray(res.results[b]["out"]) for b in range(B)])
    attn = np.stack([np.asarray(res.results[b]["attn"]) for b in range(B)])
    return (out, attn), res


def kernel(**inputs):
    (out, attn), _ = run(inputs, trace=False)
    return out, attn


# revision 7
# speedup vs baseline: 1.6507x; 1.6507x over previous
"""Trainium2 Bass kernel for nn_DisentangledHead (disentangled attention head).

Reference computation (per batch element b):
    q_c = content[b] @ w_qc ; k_c = content[b] @ w_kc ; v = content[b] @ w_v
    q_p = position @ w_qp   ; k_p = position @ w_kp
    S   = (q_c k_c^T + q_p k_p^T) * scale          [T, T]
    attn = softmax(S, -1)                           [T, T]
    out  = attn @ v                                 [T, H]
Returns (out [B,T,H], attn [B,T,T]).

Sharding: data-parallel over B across the 8 NeuronCores (1 batch element
per core; position + weights replicated).

Kernel strategy per core:
  - Build content^T / position^T via PE transposes (2 row-tiles per 128x128
    transpose).
  - Project to q_fullT / k_fullT [128, T]: partitions 0:64 hold the content
    projection transposed, 64:128 the position projection transposed, so a
    single K=128 matmul computes q_c k_c^T + q_p k_p^T.
  - Pass A (q-major): S tile [128q, T] on PE -> ACT exp(scale*S) with
    accum_out row-sums -> DVE reciprocal + tensor_scalar normalize ->
    DMA attn rows out.
  - Pass B (k-major): S^T tiles -> ACT exp -> PE accumulates
    out^T = v^T @ exp(S^T) with v stationary -> PE re-transpose ->
    DVE scale by 1/rowsum -> DMA out.
"""

import numpy as np
from contextlib import ExitStack

import concourse.bass as bass
import concourse.tile as tile
from concourse import bacc, mybir
from concourse.bass_utils import run_bass_kernel_spmd
from concourse.masks import make_identity

F32 = mybir.dt.float32
F32R = mybir.dt.float32r  # tf32 matmul mode: 4x faster PE, ~2^-11 input rounding
AF = mybir.ActivationFunctionType


def _r(ap):
    return ap.bitcast(F32R)

B = 8
T = 2048
C = 64
H = 64
P = 128
NT = T // P  # 16
NCORES = 8
SCALE = 1.0 / 8.0  # H ** -0.5

_INPUT_SPECS = [
    ("content", [T, C]),
    ("position", [T, C]),
    ("w_qc", [C, H]),
    ("w_kc", [C, H]),
    ("w_v", [C, H]),
    ("w_qp", [C, H]),
    ("w_kp", [C, H]),
]


def _emit(tc, ins, out_d, attn_d):
    nc = tc.nc
    with ExitStack() as ctx:
        consts = ctx.enter_context(tc.tile_pool(name="consts", bufs=1))
        persist = ctx.enter_context(tc.tile_pool(name="persist", bufs=1))

        ident = consts.tile([P, P], F32)
        make_identity(nc, ident)

        w_sb = {}
        w_r = {}
        for wname in ("w_qc", "w_kc", "w_v", "w_qp", "w_kp"):
            w_sb[wname] = consts.tile([C, H], F32, name=f"w_{wname}")
            nc.sync.dma_start(out=w_sb[wname], in_=ins[wname])
            w_r[wname] = consts.tile([C, H], F32R, name=f"wr_{wname}")
            nc.vector.tensor_copy(w_r[wname], w_sb[wname])

        # ---- content^T / position^T ------------------------------------
        xT = persist.tile([C, T], F32R)  # content^T (tf32-rounded)
        pT = persist.tile([C, T], F32R)  # position^T (tf32-rounded)

        with tc.tile_pool(name="tr_in", bufs=4) as tr_in, \
             tc.tile_pool(name="tr_ps", bufs=4, space="PSUM") as tr_ps:
            for src, dstT in ((ins["content"], xT), (ins["position"], pT)):
                for g in range(NT // 2):
                    nat = tr_in.tile([P, 2, C], F32)
                    # rows [256g, 256g+256) as [p, j, c] with t = j*128 + p
                    nc.sync.dma_start(
                        out=nat,
                        in_=src[2 * g * P:(2 * g + 2) * P, :].rearrange(
                            "(j p) c -> p j c", j=2))
                    pst = tr_ps.tile([P, P], F32)
                    nc.tensor.transpose(pst, nat, ident)
                    nc.vector.tensor_copy(
                        dstT[:, (2 * g) * P:(2 * g + 1) * P], pst[0:C, :])
                    nc.vector.tensor_copy(
                        dstT[:, (2 * g + 1) * P:(2 * g + 2) * P], pst[C:2 * C, :])

        # ---- projections ------------------------------------------------
        # qT rows 0:64 = (content @ w_qc)^T, rows 64:128 = (position @ w_qp)^T
        qT = persist.tile([P, T], F32R)
        kT = persist.tile([P, T], F32R)
        v_sb = persist.tile([P, NT, H], F32R)  # v rows t=128*i+p at [p, i, :]

        with tc.tile_pool(name="pj_ps", bufs=4, space="PSUM") as pj_ps:
            for n in range(4):
                sl = slice(512 * n, 512 * (n + 1))
                for wname, srcT, dst in (
                    ("w_qc", xT, qT[0:C, sl]),
                    ("w_qp", pT, qT[C:P, sl]),
                    ("w_kc", xT, kT[0:C, sl]),
                    ("w_kp", pT, kT[C:P, sl]),
                ):
                    psp = pj_ps.tile([C, 512], F32, tag="psp")
                    nc.tensor.matmul(psp, lhsT=w_r[wname], rhs=srcT[:, sl],
                                     start=True, stop=True)
                    nc.vector.tensor_copy(dst, psp)
            for i in range(NT):
                psv = pj_ps.tile([P, H], F32, tag="psv")
                nc.tensor.matmul(psv, lhsT=xT[:, P * i:P * (i + 1)],
                                 rhs=w_r["w_v"], start=True, stop=True)
                nc.vector.tensor_copy(v_sb[:, i, :], psv)

        # ---- pass A: q-major scores -> attn ----------------------------
        sums = persist.tile([P, NT], F32)
        rsum = persist.tile([P, NT], F32)

        with tc.tile_pool(name="ps_s", bufs=2, space="PSUM") as ps_s, \
             tc.tile_pool(name="attn_sb", bufs=3) as attn_pool:
            for i in range(NT):
                ps = ps_s.tile([P, T], F32)
                for n in range(4):
                    nc.tensor.matmul(
                        ps[:, 512 * n:512 * (n + 1)],
                        lhsT=qT[:, P * i:P * (i + 1)],
                        rhs=kT[:, 512 * n:512 * (n + 1)],
                        start=True, stop=True)
                et = attn_pool.tile([P, T], F32)
                nc.scalar.activation(et, ps, AF.Exp, scale=SCALE,
                                     accum_out=sums[:, i:i + 1])
                nc.vector.reciprocal(rsum[:, i:i + 1], sums[:, i:i + 1])
                nc.vector.tensor_scalar_mul(et, et, rsum[:, i:i + 1])
                nc.sync.dma_start(out=attn_d[P * i:P * (i + 1), :], in_=et)

        # ---- pass B: k-major scores -> out -----------------------------
        oT_sb = persist.tile([H, T], F32)
        with tc.tile_pool(name="ps_st", bufs=2, space="PSUM") as ps_st, \
             tc.tile_pool(name="ps_ot", bufs=1, space="PSUM") as ps_ot_pool, \
             tc.tile_pool(name="est_sb", bufs=2) as est_pool:
            ps_ot = ps_ot_pool.tile([H, T], F32)
            for j in range(NT):
                for hh in range(2):
                    ps2 = ps_st.tile([P, T // 2], F32)
                    for n in range(2):
                        qs = slice(1024 * hh + 512 * n, 1024 * hh + 512 * (n + 1))
                        nc.tensor.matmul(
                            ps2[:, 512 * n:512 * (n + 1)],
                            lhsT=kT[:, P * j:P * (j + 1)],
                            rhs=qT[:, qs], start=True, stop=True)
                    est = est_pool.tile([P, T // 2], F32R)
                    nc.scalar.activation(est, ps2, AF.Exp, scale=SCALE)
                    for n in range(2):
                        qs = slice(1024 * hh + 512 * n, 1024 * hh + 512 * (n + 1))
                        nc.tensor.matmul(
                            ps_ot[:, qs], lhsT=v_sb[:, j, :],
                            rhs=est[:, 512 * n:512 * (n + 1)],
                            start=(j == 0), stop=(j == NT - 1))
            nc.vector.tensor_copy(oT_sb, ps_ot)

        # out^T [H, T] -> out [T, H], scaled by 1/rowsum
        with tc.tile_pool(name="tr2_ps", bufs=4, space="PSUM") as tr2_ps, \
             tc.tile_pool(name="out_pool", bufs=1) as out_pool:
            out_sb = out_pool.tile([P, NT, H], F32)
            for i in range(NT):
                pst2 = tr2_ps.tile([P, H], F32)
                nc.tensor.transpose(pst2, oT_sb[:, P * i:P * (i + 1)],
                                    ident[0:C, 0:C])
                nc.vector.tensor_scalar_mul(out_sb[:, i, :], pst2,
                                            rsum[:, i:i + 1])
            out_view = out_d.rearrange("(i p) h -> p i h", p=P)
            nc.sync.dma_start(out=out_view, in_=out_sb)


def build_program():
    nc = bacc.Bacc("TRN2", target_bir_lowering=False, debug=False,
                   num_devices=NCORES)
    ins = {}
    for name, shape in _INPUT_SPECS:
        ins[name] = nc.dram_tensor(name, shape, F32, kind="ExternalInput").ap()
    attn_d = nc.dram_tensor("attn", [T, T], F32, kind="ExternalOutput").ap()
    out_d = nc.dram_tensor("out", [T, H], F32, kind="ExternalOutput").ap()
    with tile.TileContext(nc) as tc:
        _emit(tc, ins, out_d, attn_d)
    nc.compile()
    return nc


_PROGRAM = None


def _get_program():
    global _PROGRAM
    if _PROGRAM is None:
        _PROGRAM = build_program()
    return _PROGRAM


def make_in_maps(content, position, w_qc, w_kc, w_v, w_qp, w_kp):
    common = {
        "position": np.ascontiguousarray(position, dtype=np.float32),
        "w_qc": np.ascontiguousarray(w_qc, dtype=np.float32),
        "w_kc": np.ascontiguousarray(w_kc, dtype=np.float32),
        "w_v": np.ascontiguousarray(w_v, dtype=np.float32),
        "w_qp": np.ascontiguousarray(w_qp, dtype=np.float32),
        "w_kp": np.ascontiguousarray(w_kp, dtype=np.float32),
    }
    return [
        {"content": np.ascontiguousarray(content[b], dtype=np.float32), **common}
        for b in range(B)
    ]


def run(inputs, trace=False):
    nc = _get_program()
    in_maps = make_in_maps(**{k: np.asarray(v) for k, v in inputs.items()})
    res = run_bass_kernel_spmd(nc, in_maps, list(range(NCORES)), trace=trace)
    out = np.stack([np.asarray(res.results[b]["out"]) for b in range(B)])
    attn = np.stack([np.asarray(res.results[b]["attn"]) for b in range(B)])
    return (out, attn), res


def kernel(**inputs):
    (out, attn), _ = run(inputs, trace=False)
    return out, attn


# revision 11
# speedup vs baseline: 1.8680x; 1.1317x over previous
"""Trainium2 Bass kernel for nn_DisentangledHead (disentangled attention head).

Reference computation (per batch element b):
    q_c = content[b] @ w_qc ; k_c = content[b] @ w_kc ; v = content[b] @ w_v
    q_p = position @ w_qp   ; k_p = position @ w_kp
    S   = (q_c k_c^T + q_p k_p^T) * scale          [T, T]
    attn = softmax(S, -1)                           [T, T]
    out  = attn @ v                                 [T, H]
Returns (out [B,T,H], attn [B,T,T]).

Sharding: data-parallel over B across the 8 NeuronCores (1 batch element
per core; position + weights replicated).

Kernel design per core (v2 - interleaved):
  - xpT [128, T] holds [content^T ; position^T] stacked on partitions,
    built with paired PE transposes (content tile i | position tile i).
  - Block-diagonal weights [[w_qc,0],[0,w_qp]] project xpT into
    qT/kT [128, T] = [q_c^T ; q_p^T] so a single K=128 matmul computes
    q_c k_c^T + q_p k_p^T.
  - Matmuls run in float32r (tf32) - 4x the fp32 rate; operand tiles are
    float32r so producers round once.
  - Main loop interleaves, per round r: pass A (S tile [128q, T] -> ACT
    exp(scale*S) with accum_out row sums -> DVE recip + normalize -> DMA
    attn rows) and pass B (S^T half-tiles -> ACT exp -> PE accumulates
    out^T = v^T exp(S^T), v stationary, col-tiled into a [128, 1024]
    PSUM accumulator). ACT is the bottleneck engine; everything else
    overlaps under it. PSUM: 4 (S) + 2 (St) + 2 (out^T) = 8 banks.
  - Epilogue: out^T -> PE transposes -> DVE scale by 1/rowsum -> DMA out.
"""

import numpy as np
from contextlib import ExitStack

import concourse.bass as bass
import concourse.tile as tile
from concourse import bacc, mybir
from concourse.bass_utils import run_bass_kernel_spmd
from concourse.masks import make_identity

F32 = mybir.dt.float32
F32R = mybir.dt.float32r  # tf32 matmul mode: 4x faster PE, ~2^-11 input rounding
AF = mybir.ActivationFunctionType

B = 8
T = 2048
C = 64
H = 64
P = 128
NT = T // P  # 16
NCORES = 8
SCALE = 1.0 / 8.0  # H ** -0.5

_INPUT_SPECS = [
    ("content", [T, C]),
    ("position", [T, C]),
    ("w_qc", [C, H]),
    ("w_kc", [C, H]),
    ("w_v", [C, H]),
    ("w_qp", [C, H]),
    ("w_kp", [C, H]),
]


def _emit(tc, ins, out_d, attn_d):
    nc = tc.nc
    with ExitStack() as ctx:
        consts = ctx.enter_context(tc.tile_pool(name="consts", bufs=1))
        persist = ctx.enter_context(tc.tile_pool(name="persist", bufs=1))

        ident = consts.tile([P, P], F32)
        make_identity(nc, ident)

        # fp32 weight staging + block-diagonal tf32 projection weights
        w_sb = {}
        for wname in ("w_qc", "w_kc", "w_v", "w_qp", "w_kp"):
            w_sb[wname] = consts.tile([C, H], F32, name=f"w_{wname}")
            nc.sync.dma_start(out=w_sb[wname], in_=ins[wname])
        wq_stage = consts.tile([P, P], F32)
        wk_stage = consts.tile([P, P], F32)
        nc.vector.memset(wq_stage, 0.0)
        nc.vector.memset(wk_stage, 0.0)
        nc.vector.tensor_copy(wq_stage[0:C, 0:H], w_sb["w_qc"])
        nc.vector.tensor_copy(wq_stage[C:P, H:P], w_sb["w_qp"])
        nc.vector.tensor_copy(wk_stage[0:C, 0:H], w_sb["w_kc"])
        nc.vector.tensor_copy(wk_stage[C:P, H:P], w_sb["w_kp"])
        wq_blk = consts.tile([P, P], F32R)
        wk_blk = consts.tile([P, P], F32R)
        nc.vector.tensor_copy(wq_blk, wq_stage)
        nc.vector.tensor_copy(wk_blk, wk_stage)
        w_v_r = consts.tile([C, H], F32R)
        nc.vector.tensor_copy(w_v_r, w_sb["w_v"])

        # ---- xpT = [content^T ; position^T]  [128, T] --------------------
        xpT = persist.tile([P, T], F32R)
        with tc.tile_pool(name="tr_in", bufs=4) as tr_in, \
             tc.tile_pool(name="tr_ps", bufs=4, space="PSUM") as tr_ps:
            for i in range(NT):
                nat = tr_in.tile([P, 2, C], F32)
                nc.sync.dma_start(out=nat[:, 0, :],
                                  in_=ins["content"][P * i:P * (i + 1), :])
                nc.sync.dma_start(out=nat[:, 1, :],
                                  in_=ins["position"][P * i:P * (i + 1), :])
                pst = tr_ps.tile([P, P], F32)
                nc.tensor.transpose(pst, nat, ident)
                nc.vector.tensor_copy(xpT[:, P * i:P * (i + 1)], pst)

        # ---- projections -------------------------------------------------
        # qT rows 0:64 = (content @ w_qc)^T, rows 64:128 = (position @ w_qp)^T
        qT = persist.tile([P, T], F32R)
        kT = persist.tile([P, T], F32R)
        v_sb = persist.tile([P, NT, H], F32R)  # v row t=128*i+p at [p, i, :]

        with tc.tile_pool(name="pj_ps", bufs=2, space="PSUM") as pj_ps:
            for n in range(4):
                sl = slice(512 * n, 512 * (n + 1))
                psq = pj_ps.tile([P, 512], F32, tag="psq")
                nc.tensor.matmul(psq, lhsT=wq_blk, rhs=xpT[:, sl],
                                 start=True, stop=True)
                nc.vector.tensor_copy(qT[:, sl], psq)
                psk = pj_ps.tile([P, 512], F32, tag="psk")
                nc.tensor.matmul(psk, lhsT=wk_blk, rhs=xpT[:, sl],
                                 start=True, stop=True)
                nc.scalar.copy(kT[:, sl], psk)
            for i in range(NT):
                psv = pj_ps.tile([P, H], F32, tag="psv")
                nc.tensor.matmul(psv, lhsT=xpT[0:C, P * i:P * (i + 1)],
                                 rhs=w_v_r, start=True, stop=True)
                nc.vector.tensor_copy(v_sb[:, i, :], psv)

        # ---- interleaved main loop --------------------------------------
        sums2 = persist.tile([P, NT, 2], F32)
        sums = persist.tile([P, NT], F32)
        rsum = persist.tile([P, NT], F32)
        oT_sb = persist.tile([H, T], F32)
        HT = T // 2

        with tc.tile_pool(name="ps_work", bufs=2, space="PSUM") as ps_work, \
             tc.tile_pool(name="ps_ot", bufs=1, space="PSUM") as ps_ot_pool, \
             tc.tile_pool(name="attn_sb", bufs=3) as attn_pool, \
             tc.tile_pool(name="est_sb", bufs=2) as est_pool:
            ps_ot = ps_ot_pool.tile([H, T], F32)
            for r in range(NT):
                rq = slice(P * r, P * (r + 1))
                et = attn_pool.tile([P, T], F32)
                # pass A: S row-block r, two half-tiles
                for hh in range(2):
                    ks = slice(HT * hh, HT * (hh + 1))
                    ps = ps_work.tile([P, HT], F32, tag="work")
                    for n in range(2):
                        nc.tensor.matmul(
                            ps[:, 512 * n:512 * (n + 1)],
                            lhsT=qT[:, rq],
                            rhs=kT[:, HT * hh + 512 * n:HT * hh + 512 * (n + 1)],
                            start=True, stop=True)
                    nc.scalar.activation(et[:, ks], ps, AF.Exp, scale=SCALE,
                                         accum_out=sums2[:, r, hh:hh + 1])
                nc.vector.tensor_add(sums[:, r:r + 1], sums2[:, r, 0:1],
                                     sums2[:, r, 1:2])
                nc.vector.reciprocal(rsum[:, r:r + 1], sums[:, r:r + 1])

                # pass B: S^T col-block r, two half-tiles; accumulate out^T
                for hh in range(2):
                    ps2 = ps_work.tile([P, HT], F32, tag="work")
                    for n in range(2):
                        qs = slice(HT * hh + 512 * n, HT * hh + 512 * (n + 1))
                        nc.tensor.matmul(
                            ps2[:, 512 * n:512 * (n + 1)],
                            lhsT=kT[:, rq],
                            rhs=qT[:, qs], start=True, stop=True)
                    est = est_pool.tile([P, HT], F32R)
                    nc.scalar.activation(est, ps2, AF.Exp, scale=SCALE)
                    for n in range(2):
                        nc.tensor.matmul(
                            ps_ot[:, HT * hh + 512 * n:HT * hh + 512 * (n + 1)],
                            lhsT=v_sb[:, r, :],
                            rhs=est[:, 512 * n:512 * (n + 1)],
                            start=(r == 0), stop=(r == NT - 1))

                nc.vector.tensor_scalar_mul(et, et, rsum[:, r:r + 1])
                nc.sync.dma_start(out=attn_d[P * r:P * (r + 1), :], in_=et)
            nc.vector.tensor_copy(oT_sb, ps_ot)

        # ---- epilogue: out^T -> out, scaled by 1/rowsum ------------------
        with tc.tile_pool(name="tr2_ps", bufs=4, space="PSUM") as tr2_ps, \
             tc.tile_pool(name="out_pool", bufs=1) as out_pool:
            out_sb = out_pool.tile([P, NT, H], F32)
            for i in range(NT):
                pso = tr2_ps.tile([P, H], F32)
                nc.tensor.transpose(pso, oT_sb[:, P * i:P * (i + 1)],
                                    ident[0:H, 0:H])
                nc.vector.tensor_scalar_mul(out_sb[:, i, :], pso,
                                            rsum[:, i:i + 1])
            out_view = out_d.rearrange("(i p) h -> p i h", p=P)
            nc.sync.dma_start(out=out_view, in_=out_sb)


def build_program():
    nc = bacc.Bacc("TRN2", target_bir_lowering=False, debug=False,
                   num_devices=NCORES)
    ins = {}
    for name, shape in _INPUT_SPECS:
        ins[name] = nc.dram_tensor(name, shape, F32, kind="ExternalInput").ap()
    attn_d = nc.dram_tensor("attn", [T, T], F32, kind="ExternalOutput").ap()
    out_d = nc.dram_tensor("out", [T, H], F32, kind="ExternalOutput").ap()
    with tile.TileContext(nc) as tc:
        _emit(tc, ins, out_d, attn_d)
    nc.compile()
    return nc


_PROGRAM = None


def _get_program():
    global _PROGRAM
    if _PROGRAM is None:
        _PROGRAM = build_program()
    return _PROGRAM


def make_in_maps(content, position, w_qc, w_kc, w_v, w_qp, w_kp):
    common = {
        "position": np.ascontiguousarray(position, dtype=np.float32),
        "w_qc": np.ascontiguousarray(w_qc, dtype=np.float32),
        "w_kc": np.ascontiguousarray(w_kc, dtype=np.float32),
        "w_v": np.ascontiguousarray(w_v, dtype=np.float32),
        "w_qp": np.ascontiguousarray(w_qp, dtype=np.float32),
        "w_kp": np.ascontiguousarray(w_kp, dtype=np.float32),
    }
    return [
        {"content": np.ascontiguousarray(content[b], dtype=np.float32), **common}
        for b in range(B)
    ]


def run(inputs, trace=False):
    nc = _get_program()
    in_maps = make_in_maps(**{k: np.asarray(v) for k, v in inputs.items()})
    res = run_bass_kernel_spmd(nc, in_maps, list(range(NCORES)), trace=trace)
    out = np.stack([np.asarray(res.results[b]["out"]) for b in range(B)])
    attn = np.stack([np.asarray(res.results[b]["attn"]) for b in range(B)])
    return (out, attn), res


def kernel(**inputs):
    (out, attn), _ = run(inputs, trace=False)
    return out, attn


# revision 13
# speedup vs baseline: 2.1197x; 1.1348x over previous
"""Trainium2 Bass kernel for nn_DisentangledHead (disentangled attention head).

Reference computation (per batch element b):
    q_c = content[b] @ w_qc ; k_c = content[b] @ w_kc ; v = content[b] @ w_v
    q_p = position @ w_qp   ; k_p = position @ w_kp
    S   = (q_c k_c^T + q_p k_p^T) * scale          [T, T]
    attn = softmax(S, -1)                           [T, T]
    out  = attn @ v                                 [T, H]
Returns (out [B,T,H], attn [B,T,T]).

Sharding: data-parallel over B across the 8 NeuronCores (1 batch element
per core; position + weights replicated).

Kernel design per core (v2 - interleaved):
  - xpT [128, T] holds [content^T ; position^T] stacked on partitions,
    built with paired PE transposes (content tile i | position tile i).
  - Block-diagonal weights [[w_qc,0],[0,w_qp]] project xpT into
    qT/kT [128, T] = [q_c^T ; q_p^T] so a single K=128 matmul computes
    q_c k_c^T + q_p k_p^T.
  - Matmuls run in float32r (tf32) - 4x the fp32 rate; operand tiles are
    float32r so producers round once.
  - Main loop interleaves, per round r: pass A (S tile [128q, T] -> ACT
    exp(scale*S) with accum_out row sums -> DVE recip + normalize -> DMA
    attn rows) and pass B (S^T half-tiles -> ACT exp -> PE accumulates
    out^T = v^T exp(S^T), v stationary, col-tiled into a [128, 1024]
    PSUM accumulator). ACT is the bottleneck engine; everything else
    overlaps under it. PSUM: 4 (S) + 2 (St) + 2 (out^T) = 8 banks.
  - Epilogue: out^T -> PE transposes -> DVE scale by 1/rowsum -> DMA out.
"""

import numpy as np
from contextlib import ExitStack

import concourse.bass as bass
import concourse.tile as tile
from concourse import bacc, mybir
from concourse.bass_utils import run_bass_kernel_spmd
from concourse.masks import make_identity

F32 = mybir.dt.float32
F32R = mybir.dt.float32r  # tf32 matmul mode: 4x faster PE, ~2^-11 input rounding
AF = mybir.ActivationFunctionType

B = 8
T = 2048
C = 64
H = 64
P = 128
NT = T // P  # 16
NCORES = 8
SCALE = 1.0 / 8.0  # H ** -0.5

_INPUT_SPECS = [
    ("content", [T, C]),
    ("position", [T, C]),
    ("w_qc", [C, H]),
    ("w_kc", [C, H]),
    ("w_v", [C, H]),
    ("w_qp", [C, H]),
    ("w_kp", [C, H]),
]


def _emit(tc, ins, out_d, attn_d):
    nc = tc.nc
    with ExitStack() as ctx:
        consts = ctx.enter_context(tc.tile_pool(name="consts", bufs=1))
        persist = ctx.enter_context(tc.tile_pool(name="persist", bufs=1))

        ident = consts.tile([P, P], F32)
        make_identity(nc, ident)

        # fp32 weight staging + block-diagonal tf32 projection weights
        w_sb = {}
        for wname in ("w_qc", "w_kc", "w_v", "w_qp", "w_kp"):
            w_sb[wname] = consts.tile([C, H], F32, name=f"w_{wname}")
            nc.gpsimd.dma_start(out=w_sb[wname], in_=ins[wname])
        wq_stage = consts.tile([P, P], F32)
        wk_stage = consts.tile([P, P], F32)
        nc.vector.memset(wq_stage, 0.0)
        nc.vector.memset(wk_stage, 0.0)
        nc.vector.tensor_copy(wq_stage[0:C, 0:H], w_sb["w_qc"])
        nc.vector.tensor_copy(wq_stage[C:P, H:P], w_sb["w_qp"])
        nc.vector.tensor_copy(wk_stage[0:C, 0:H], w_sb["w_kc"])
        nc.vector.tensor_copy(wk_stage[C:P, H:P], w_sb["w_kp"])
        wq_blk = consts.tile([P, P], F32R)
        wk_blk = consts.tile([P, P], F32R)
        nc.vector.tensor_copy(wq_blk, wq_stage)
        nc.vector.tensor_copy(wk_blk, wk_stage)
        w_v_r = consts.tile([C, H], F32R)
        nc.vector.tensor_copy(w_v_r, w_sb["w_v"])

        # ---- xpT = [content^T ; position^T]  [128, T] --------------------
        # Stage both inputs with one DMA each: stage[p, 0, i, c] = content
        # row 128*i+p, stage[p, 1, i, c] = position row 128*i+p.
        xpT = persist.tile([P, T], F32R)
        with tc.tile_pool(name="tr_in", bufs=1) as tr_in, \
             tc.tile_pool(name="tr_ps", bufs=4, space="PSUM") as tr_ps:
            stage = tr_in.tile([P, NT, 2, C], F32)
            nc.sync.dma_start(
                out=stage[:, :, 0, :],
                in_=ins["content"].rearrange("(i p) c -> p i c", p=P))
            nc.sync.dma_start(
                out=stage[:, :, 1, :],
                in_=ins["position"].rearrange("(i p) c -> p i c", p=P))
            for i in range(NT):
                pst = tr_ps.tile([P, P], F32)
                nc.tensor.transpose(pst, stage[:, i, :, :], ident)
                nc.vector.tensor_copy(xpT[:, P * i:P * (i + 1)], pst)

        # ---- projections -------------------------------------------------
        # qT rows 0:64 = (content @ w_qc)^T, rows 64:128 = (position @ w_qp)^T
        qT = persist.tile([P, T], F32R)
        kT = persist.tile([P, T], F32R)
        v_sb = persist.tile([P, NT, H], F32R)  # v row t=128*i+p at [p, i, :]

        with tc.tile_pool(name="pj_ps", bufs=2, space="PSUM") as pj_ps:
            for n in range(4):
                sl = slice(512 * n, 512 * (n + 1))
                psq = pj_ps.tile([P, 512], F32, tag="psq")
                nc.tensor.matmul(psq, lhsT=wq_blk, rhs=xpT[:, sl],
                                 start=True, stop=True)
                nc.vector.tensor_copy(qT[:, sl], psq)
                psk = pj_ps.tile([P, 512], F32, tag="psk")
                nc.tensor.matmul(psk, lhsT=wk_blk, rhs=xpT[:, sl],
                                 start=True, stop=True)
                nc.scalar.copy(kT[:, sl], psk)
            for i in range(NT):
                psv = pj_ps.tile([P, H], F32, tag="psv")
                nc.tensor.matmul(psv, lhsT=xpT[0:C, P * i:P * (i + 1)],
                                 rhs=w_v_r, start=True, stop=True)
                nc.vector.tensor_copy(v_sb[:, i, :], psv)

        # ---- interleaved main loop --------------------------------------
        sums2 = persist.tile([P, NT, 2], F32)
        sums = persist.tile([P, NT], F32)
        rsum = persist.tile([P, NT], F32)
        oT_sb = persist.tile([H, T], F32)
        HT = T // 2

        with tc.tile_pool(name="ps_work", bufs=2, space="PSUM") as ps_work, \
             tc.tile_pool(name="ps_ot", bufs=1, space="PSUM") as ps_ot_pool, \
             tc.tile_pool(name="attn_sb", bufs=3) as attn_pool, \
             tc.tile_pool(name="est_sb", bufs=2) as est_pool:
            ps_ot = ps_ot_pool.tile([H, T], F32)
            for r in range(NT):
                rq = slice(P * r, P * (r + 1))
                et = attn_pool.tile([P, T], F32)
                # pass A: S row-block r, two half-tiles
                for hh in range(2):
                    ks = slice(HT * hh, HT * (hh + 1))
                    ps = ps_work.tile([P, HT], F32, tag="work")
                    for n in range(2):
                        nc.tensor.matmul(
                            ps[:, 512 * n:512 * (n + 1)],
                            lhsT=qT[:, rq],
                            rhs=kT[:, HT * hh + 512 * n:HT * hh + 512 * (n + 1)],
                            start=True, stop=True)
                    nc.scalar.activation(et[:, ks], ps, AF.Exp, scale=SCALE,
                                         accum_out=sums2[:, r, hh:hh + 1])
                nc.vector.tensor_add(sums[:, r:r + 1], sums2[:, r, 0:1],
                                     sums2[:, r, 1:2])
                nc.vector.reciprocal(rsum[:, r:r + 1], sums[:, r:r + 1])

                # pass B: S^T col-block r, two half-tiles; accumulate out^T
                for hh in range(2):
                    ps2 = ps_work.tile([P, HT], F32, tag="work")
                    for n in range(2):
                        qs = slice(HT * hh + 512 * n, HT * hh + 512 * (n + 1))
                        nc.tensor.matmul(
                            ps2[:, 512 * n:512 * (n + 1)],
                            lhsT=kT[:, rq],
                            rhs=qT[:, qs], start=True, stop=True)
                    est = est_pool.tile([P, HT], F32R)
                    nc.scalar.activation(est, ps2, AF.Exp, scale=SCALE)
                    for n in range(2):
                        nc.tensor.matmul(
                            ps_ot[:, HT * hh + 512 * n:HT * hh + 512 * (n + 1)],
                            lhsT=v_sb[:, r, :],
                            rhs=est[:, 512 * n:512 * (n + 1)],
                            start=(r == 0), stop=(r == NT - 1))

                nc.vector.tensor_scalar_mul(et, et, rsum[:, r:r + 1])
                nc.sync.dma_start(out=attn_d[P * r:P * (r + 1), :], in_=et)
            nc.vector.tensor_copy(oT_sb, ps_ot)

        # ---- epilogue: out^T -> out, scaled by 1/rowsum ------------------
        with tc.tile_pool(name="tr2_ps", bufs=4, space="PSUM") as tr2_ps, \
             tc.tile_pool(name="out_pool", bufs=1) as out_pool:
            out_sb = out_pool.tile([P, NT, H], F32)
            for i in range(NT):
                pso = tr2_ps.tile([P, H], F32)
                nc.tensor.transpose(pso, oT_sb[:, P * i:P * (i + 1)],
                                    ident[0:H, 0:H])
                nc.vector.tensor_scalar_mul(out_sb[:, i, :], pso,
                                            rsum[:, i:i + 1])
            out_view = out_d.rearrange("(i p) h -> p i h", p=P)
            nc.sync.dma_start(out=out_view, in_=out_sb)


def build_program():
    nc = bacc.Bacc("TRN2", target_bir_lowering=False, debug=False,
                   num_devices=NCORES)
    ins = {}
    for name, shape in _INPUT_SPECS:
        ins[name] = nc.dram_tensor(name, shape, F32, kind="ExternalInput").ap()
    attn_d = nc.dram_tensor("attn", [T, T], F32, kind="ExternalOutput").ap()
    out_d = nc.dram_tensor("out", [T, H], F32, kind="ExternalOutput").ap()
    with tile.TileContext(nc) as tc:
        _emit(tc, ins, out_d, attn_d)
    nc.compile()
    return nc


_PROGRAM = None


def _get_program():
    global _PROGRAM
    if _PROGRAM is None:
        _PROGRAM = build_program()
    return _PROGRAM


def make_in_maps(content, position, w_qc, w_kc, w_v, w_qp, w_kp):
    common = {
        "position": np.ascontiguousarray(position, dtype=np.float32),
        "w_qc": np.ascontiguousarray(w_qc, dtype=np.float32),
        "w_kc": np.ascontiguousarray(w_kc, dtype=np.float32),
        "w_v": np.ascontiguousarray(w_v, dtype=np.float32),
        "w_qp": np.ascontiguousarray(w_qp, dtype=np.float32),
        "w_kp": np.ascontiguousarray(w_kp, dtype=np.float32),
    }
    return [
        {"content": np.ascontiguousarray(content[b], dtype=np.float32), **common}
        for b in range(B)
    ]


def run(inputs, trace=False):
    nc = _get_program()
    in_maps = make_in_maps(**{k: np.asarray(v) for k, v in inputs.items()})
    res = run_bass_kernel_spmd(nc, in_maps, list(range(NCORES)), trace=trace)
    out = np.stack([np.asarray(res.results[b]["out"]) for b in range(B)])
    attn = np.stack([np.asarray(res.results[b]["attn"]) for b in range(B)])
    return (out, attn), res


def kernel(**inputs):
    (out, attn), _ = run(inputs, trace=False)
    return out, attn
